# revision 37
# baseline (speedup 1.0000x reference)
"""Multi-headed attention (B=2, S=2048, D=1024, H=16) on 8 TRN2 NeuronCores.

Sharding: tensor-parallel over heads for the attention body (2 heads/core,
both batches on every core), then AllToAll reshards to (batch, seq-quarter)
for the output projection. Per core:

  1. K/V/Q projections (bf16 matmuls, fp32 psum):
       qhT/khT [128e, 2048s] (e on partitions), vh [2048t, 128e'].
  2. logits^T = khT-tiles.T @ qhT  (K=64, two heads row-packed: head0 ->
     psum bank A, head1 -> bank B of one [128,1024] tile).
  3. P = exp(0.125 * logits^T) on ScalarE (PSUM -> SBUF bf16, FD=1024).
  4. heads^T += vht_ext.T @ P where vht_ext = [e | ones | e] (192 cols):
     head0 lhsT = cols 0:128 -> A0 = [AV0 | rowsum0-replicas]; head1
     lhsT = cols 64:192 -> A1 = [rowsum1-replicas | AV1]. The folded
     ones columns make the softmax denominators fall out of the same
     matmul -- no separate rowsum matmuls (-23% tensor-engine work vs
     computing ones.T @ P separately).
  5. rec = reciprocal_approx_fast(full banks; the AV-row reciprocals are
     discarded -- the custom-DVE op miscomputes on HW when operands have
     a partition offset), partition-rotate the rowsum halves onto the
     AV-aligned lanes via SBUF DMA, then heads^T *= rec -> hN bf16.
  6. Two AllToAlls (one per batch, zero-padded blocks for the other batch's
     ranks), fired as each batch finishes. The receiver sums the two outputs
     (one is zeros for this rank), so no data-dependent branching is needed.
  7. out = gelu_sigmoid(heads_full^T-tiles.T @ Wo + bo) -> [512, 1024] f32
     = (batch r//4, seq-quarter r%4) slab of the full output. The batch-0
     half of the contraction is emitted BEFORE the second AllToAll's
     trigger so it fills that collective's window (sync-queue DMAs emitted
     after a collective trigger gate on its completion, so hf1 gates only
     on AllToAll#1; the PE itself is never collective-gated).

Batch-1 input DMAs are dripped into batch-0's attention loop as prefetch;
the batch-1 projection matmuls run as blocks between phases, sharing the
A0/A1 psum slots (PSUM: A0 x2 + A1 x2 + L x2x2 = 8 banks).
"""

import numpy as np
import ml_dtypes

import concourse.bass as bass
import concourse.mybir as mybir
import concourse.tile as tile
from concourse import bacc
from concourse.bass_utils import run_bass_kernel_spmd

F = mybir.ActivationFunctionType
BF16 = mybir.dt.bfloat16
F32 = mybir.dt.float32
BF = ml_dtypes.bfloat16
GELU = F.Gelu_apprx_sigmoid  # sim_check overrides (interp lacks this fn)
DEBUG_DUMPS = False

B, S, D, H = 2, 2048, 1024, 16
HD = D // H
NCORES = 8
SQ = S // 4
KT = D // 128
TT = S // 128
SC = S // 512

_CACHE = {}


def _build():
    nc = bacc.Bacc("TRN2", target_bir_lowering=False, debug=False,
                   num_devices=NCORES)
    xq = [nc.dram_tensor(f"xq{b}", [D, S], BF16, kind="ExternalInput") for b in range(B)]
    xk = [nc.dram_tensor(f"xk{b}", [D, S], BF16, kind="ExternalInput") for b in range(B)]
    xv = [nc.dram_tensor(f"xv{b}", [D, S], BF16, kind="ExternalInput") for b in range(B)]
    wq_d = nc.dram_tensor("wq", [D, 128], BF16, kind="ExternalInput")
    wk_d = nc.dram_tensor("wk", [D, 128], BF16, kind="ExternalInput")
    wv_d = nc.dram_tensor("wv", [D, 128], BF16, kind="ExternalInput")
    bq_d = nc.dram_tensor("bq", [128, 1], F32, kind="ExternalInput")
    bk_d = nc.dram_tensor("bk", [128, 1], F32, kind="ExternalInput")
    bv_d = nc.dram_tensor("bv", [1, 128], BF16, kind="ExternalInput")
    wo_d = nc.dram_tensor("wo", [D, D], BF16, kind="ExternalInput")
    bo_d = nc.dram_tensor("bo", [1, D], BF16, kind="ExternalInput")
    onr_d = nc.dram_tensor("onr", [1, 128], BF16, kind="ExternalInput")
    out_d = nc.dram_tensor("out", [SQ, D], F32, kind="ExternalOutput")
    if DEBUG_DUMPS:
        dbg_vht = nc.dram_tensor("dbg_vht", [128, 192], BF16, kind="ExternalOutput")
        dbg_p = nc.dram_tensor("dbg_p", [128, 1024], BF16, kind="ExternalOutput")
        dbg_rcs = nc.dram_tensor("dbg_rcs", [128, 512], F32, kind="ExternalOutput")
        dbg_rec = nc.dram_tensor("dbg_rec", [128, 512], F32, kind="ExternalOutput")
        dbg_hn = nc.dram_tensor("dbg_hn", [128, 512], BF16, kind="ExternalOutput")

    xqr = [xq[b][:, :].rearrange("(kt p) s -> kt p s", p=128) for b in range(B)]
    xkr = [xk[b][:, :].rearrange("(kt p) s -> kt p s", p=128) for b in range(B)]
    xvr = [xv[b][:, :].rearrange("(kt p) s -> kt p s", p=128) for b in range(B)]

    with tile.TileContext(nc) as tc:
        with tc.tile_pool(name="cst", bufs=1) as cst, \
             tc.tile_pool(name="act", bufs=1) as acp, \
             tc.tile_pool(name="str", bufs=4) as stp, \
             tc.tile_pool(name="s2", bufs=3) as s2p, \
             tc.tile_pool(name="ps", bufs=2, space="PSUM") as ps, \
             tc.tile_pool(name="dram", bufs=1, space="DRAM") as dp:

            # small weights/biases first so the first projection can start
            wqt = cst.tile([128, KT, 128], BF16, tag="wqt")
            wkt = cst.tile([128, KT, 128], BF16, tag="wkt")
            wvt = cst.tile([128, KT, 128], BF16, tag="wvt")
            # only wkt up front; wqt/wvt are queued after the first xk chunks
            # in the schedule so the first projection starts ~3us earlier
            nc.sync.dma_start(wkt[:, :, :], wk_d[:, :].rearrange("(kt p) e -> p kt e", p=128))
            bqt = cst.tile([128, 1], F32, tag="bqt")
            bkt = cst.tile([128, 1], F32, tag="bkt")
            bvt = cst.tile([1, 128], BF16, tag="bvt")
            bot = cst.tile([1, D], BF16, tag="bot")
            onr = cst.tile([1, 128], BF16, tag="onr")
            # scalar queue: keeps the sync queue clear for the first xk
            # chunks (each queued descriptor costs ~0.5us of startup latency)
            for t, d in ((bkt, bk_d), (bqt, bq_d), (bvt, bv_d), (bot, bo_d),
                         (onr, onr_d)):
                nc.scalar.dma_start(t[:, :], d[:, :])
            zt = cst.tile([128, SQ], BF16, tag="zt")
            nc.vector.memset(zt[:, :], 0.0)

            qhT = [acp.tile([128, S], BF16, tag=f"qhT{b}", name=f"qhT{b}") for b in range(B)]
            khT = [acp.tile([128, S], BF16, tag=f"khT{b}", name=f"khT{b}") for b in range(B)]
            # vht_ext: cols 0:64 head0 e-vals, 64:128 ones, 128:192 head1.
            # head0 AV lhsT = cols 0:128 (AV rows 0:64, rowsum rows 64:128);
            # head1 AV lhsT = cols 64:192 (rowsum rows 0:64, AV rows 64:128).
            vht = [acp.tile([128, TT, 192], BF16, tag=f"vht{b}", name=f"vht{b}") for b in range(B)]
            for b in range(B):
                for tt in range(TT):  # per-tile 2D memsets: strided 3D memset
                    nc.vector.memset(vht[b][:, tt, 64:128], 1.0)  # miswrites on HW
            # one shared slot: vx[1] reuses vx[0]'s space once vproj(0) is done
            vx = [acp.tile([128, KT, S], BF16, tag="vx", name=f"vx{b}") for b in range(B)]
            hN = [acp.tile([128, S], BF16, tag=f"hN{b}", name=f"hN{b}") for b in range(B)]
            wot = cst.tile([128, KT, D], BF16, tag="wot")

            a2a_in = [dp.tile([NCORES, 128, SQ], BF16, tag=f"a2a_in{b}", name=f"a2a_in{b}")
                      for b in range(B)]
            a2a_out = [dp.tile([NCORES, 128, SQ], BF16, tag=f"a2a_out{b}", name=f"a2a_out{b}")
                       for b in range(B)]
            def emit_a2a_padding():
                # deferred: these pads are only read by the collectives, but
                # if emitted up front they delay the vx loads on the gpsimd
                # queue by ~4us
                for b in range(B):
                    for r in range(NCORES):
                        if r // 4 != b:
                            nc.gpsimd.dma_start(a2a_in[b][r, :, :], zt[:, :])

            # ---------- emission helpers ----------
            # PSUM map (all tags double-buffered; single-buffered PSUM reuse
            # races on HW): A0 2 banks + A1 2 banks + L 4 banks = 8.
            # Projections borrow the A0/A1 slots between attention phases.
            xcs = {}

            def kqproj_load(b, which, sp):
                """DMA-prefetch of the x chunks for one projection half."""
                xr, pre = {"k": (xkr[b], "xk"), "q": (xqr[b], "xq")}[which]
                lst = []
                for kt in range(KT):
                    def mk(kt=kt):
                        xc = stp.tile([128, 1024], BF16, tag=pre, bufs=9,
                                      name=f"{pre}{b}{sp}{kt}")
                        nc.sync.dma_start(xc[:, :],
                                          xr[kt, :, sp * 1024:(sp + 1) * 1024])
                        lst.append(xc)
                    yield mk
                xcs[(b, which, sp)] = lst

            def kqproj_mm(b, which, sp):
                """Projection matmuls + bias for one 1024-wide s-half."""
                w_t, b_t, dst = {
                    "k": (wkt, bkt, khT[b]),
                    "q": (wqt, bqt, qhT[b]),
                }[which]
                xc = xcs[(b, which, sp)]
                for half in range(2):
                    P = ps.tile([128, 512], F32, tag="A0",
                                name=f"{which}p{b}{sp}{half}")
                    for kt in range(KT):
                        nc.tensor.matmul(P[:, :], w_t[:, kt, :],
                                         xc[kt][:, half * 512:(half + 1) * 512],
                                         start=(kt == 0), stop=(kt == KT - 1))
                    off = sp * 1024 + half * 512
                    nc.vector.tensor_scalar_add(dst[:, off:off + 512],
                                                P[:, :], b_t[:, 0:1])

            def vload_steps(b):
                for kt in range(KT):
                    def mk(b=b, kt=kt):
                        nc.gpsimd.dma_start(vx[b][:, kt, :], xvr[b][kt, :, :])
                    yield mk

            def vproj_mm(b):
                for tt in range(TT):
                    Vp = ps.tile([128, 128], F32, tag="A1", name=f"Vp{b}{tt}")
                    for kt in range(KT):
                        nc.tensor.matmul(Vp[:, :],
                                         vx[b][:, kt, tt * 128:(tt + 1) * 128],
                                         wvt[:, kt, :], start=(kt == 0), stop=False)
                    nc.tensor.matmul(Vp[:, :], onr[0:1, :], bvt[0:1, :],
                                     start=False, stop=True)
                    nc.vector.tensor_copy(vht[b][:, tt, 0:64], Vp[:, 0:64])
                    nc.vector.tensor_copy(vht[b][:, tt, 128:192], Vp[:, 64:128])

            def stage2(b, sc, filler=None):
                s0, s1 = sc * 512, (sc + 1) * 512
                A0 = ps.tile([128, 512], F32, tag="A0", name=f"A0_{b}{sc}")
                A1 = ps.tile([128, 512], F32, tag="A1", name=f"A1_{b}{sc}")
                for tt in range(TT):
                    t0, t1 = tt * 128, (tt + 1) * 128
                    L2 = ps.tile([128, 1024], F32, tag="L", name=f"L2{b}{sc}{tt}")
                    nc.tensor.matmul(L2[:, 0:512], khT[b][0:64, t0:t1],
                                     qhT[b][0:64, s0:s1], start=True, stop=True)
                    nc.tensor.matmul(L2[:, 512:1024], khT[b][64:128, t0:t1],
                                     qhT[b][64:128, s0:s1], start=True, stop=True)
                    P = s2p.tile([128, 1024], BF16, tag="P", bufs=4, name=f"P{b}{sc}{tt}")
                    nc.scalar.activation(P[:, :], L2[:, :], F.Exp, scale=0.125)
                    if DEBUG_DUMPS and b == 0 and sc == 1 and tt == 0:
                        nc.sync.dma_start(dbg_p[:, :], P[:, :])
                    st, sp_ = (tt == 0), (tt == TT - 1)
                    nc.tensor.matmul(A0[:, :], vht[b][:, tt, 0:128], P[:, 0:512],
                                     start=st, stop=sp_)
                    nc.tensor.matmul(A1[:, :], vht[b][:, tt, 64:192], P[:, 512:1024],
                                     start=st, stop=sp_)
                    if filler is not None:
                        step = next(filler, None)
                        if step is not None:
                            step()
                # rowsum replicas live on the opposite 64-partition half from
                # their AV rows. reciprocal_approx_fast miscomputes when its
                # operands have a partition offset (HW-only bug), so recip the
                # FULL banks (AV rows give garbage reciprocals we discard),
                # then partition-rotate the useful halves via SBUF DMA.
                rca = s2p.tile([128, 512], F32, tag="rca", bufs=2, name=f"rca{b}{sc}")
                nc.vector.reciprocal_approx_fast(rca[:, :], A0[:, :])
                rcb = s2p.tile([128, 512], F32, tag="rcb", bufs=2, name=f"rcb{b}{sc}")
                nc.vector.reciprocal_approx_fast(rcb[:, :], A1[:, :])
                # gpsimd queue: sync-queue DMAs emitted after a collective
                # trigger are gated on collective completion; gpsimd triggers
                # stay ahead of the collective trigger in their queue.
                rec = s2p.tile([128, 512], F32, tag="rec", bufs=2, name=f"rec{b}{sc}")
                nc.gpsimd.dma_start(rec[0:64, :], rca[64:128, :])
                nc.gpsimd.dma_start(rec[64:128, :], rcb[0:64, :])
                nc.vector.tensor_mul(hN[b][0:64, s0:s1], A0[0:64, :], rec[0:64, :])
                nc.vector.tensor_mul(hN[b][64:128, s0:s1], A1[64:128, :], rec[64:128, :])
                if DEBUG_DUMPS and b == 0 and sc == 1:
                    nc.sync.dma_start(dbg_vht[:, :], vht[0][:, 0, :])
                    nc.sync.dma_start(dbg_rcs[:, :], rca[:, :])
                    nc.sync.dma_start(dbg_rec[:, :], rec[:, :])
                    nc.sync.dma_start(dbg_hn[:, :], hN[b][:, s0:s1])
                nc.sync.dma_start(a2a_in[b][4 * b + sc, :, :], hN[b][:, s0:s1])

            # ---------- schedule ----------
            import itertools
            for step in itertools.chain(vload_steps(0),
                                        kqproj_load(0, "k", 0)):
                step()
            nc.sync.dma_start(wvt[:, :, :], wv_d[:, :].rearrange("(kt p) e -> p kt e", p=128))
            nc.sync.dma_start(wqt[:, :, :], wq_d[:, :].rearrange("(kt p) e -> p kt e", p=128))
            for step in kqproj_load(0, "k", 1):
                step()
            kqproj_mm(0, "k", 0)
            kqproj_mm(0, "k", 1)
            for step in itertools.chain(kqproj_load(0, "q", 0),
                                        kqproj_load(0, "q", 1)):
                step()
            # q before v: the q bias-adds must clear the DVE queue before the
            # 32 vht copies, else stage2's first logits stall on qhT
            kqproj_mm(0, "q", 0)
            kqproj_mm(0, "q", 1)
            vproj_mm(0)

            # batch-1 input DMAs dripped into batch-0 attention (prefetch
            # only; the projection matmuls run as blocks between phases so
            # the A0/A1 psum slots stay phase-ordered).
            fillerA = itertools.chain(vload_steps(1),
                                      kqproj_load(1, "k", 0),
                                      kqproj_load(1, "k", 1),
                                      kqproj_load(1, "q", 0),
                                      kqproj_load(1, "q", 1))
            stage2(0, 0, fillerA)
            emit_a2a_padding()
            # wot load well before any collective trigger: sync-queue DMAs
            # emitted after a collective trigger gate on its completion.
            nc.sync.dma_start(wot[:, :, :],
                              wo_d[:, :].rearrange("(kt p) n -> p kt n", p=128))
            stage2(0, 1, fillerA)
            stage2(0, 2, fillerA)
            stage2(0, 3, fillerA)
            for step in fillerA:
                step()
            nc.gpsimd.collective_compute(
                "AllToAll", mybir.AluOpType.bypass,
                replica_groups=[list(range(NCORES))],
                ins=[a2a_in[0].opt()], outs=[a2a_out[0].opt()])
            kqproj_mm(1, "k", 0)
            kqproj_mm(1, "k", 1)
            kqproj_mm(1, "q", 0)
            vproj_mm(1)
            stage2(1, 0)
            stage2(1, 1)
            kqproj_mm(1, "q", 1)
            stage2(1, 2)
            stage2(1, 3)

            # ---- batch-0 output projection emitted BEFORE the second
            # AllToAll trigger: its hf1 DMAs gate only on AllToAll#1 (long
            # done) and the PE is not collective-gated, so this work fills
            # the second collective's window (no dummy warmup needed).
            hf1 = acp.tile([128, NCORES, SQ], BF16, tag="hf1")
            for p in range(NCORES):
                nc.sync.dma_start(hf1[:, p, :], a2a_out[0][p, :, :])
            o1 = acp.tile([128, 4, D], BF16, tag="o1")
            for st in range(4):
                O = ps.tile([128, 1024], F32, tag="L", name=f"O1_{st}")
                for nn in range(2):
                    n0, n1 = nn * 512, (nn + 1) * 512
                    for kt in range(KT):
                        nc.tensor.matmul(O[:, n0:n1],
                                         hf1[:, kt, st * 128:(st + 1) * 128],
                                         wot[:, kt, n0:n1],
                                         start=(kt == 0), stop=False)
                    nc.tensor.matmul(O[:, n0:n1], onr[0:1, :], bot[0:1, n0:n1],
                                     start=False, stop=True)
                nc.vector.tensor_copy(o1[:, st, :], O[:, :])

            nc.gpsimd.collective_compute(
                "AllToAll", mybir.AluOpType.bypass,
                replica_groups=[list(range(NCORES))],
                ins=[a2a_in[1].opt()], outs=[a2a_out[1].opt()])

            hf2 = acp.tile([128, NCORES, SQ], BF16, tag="hf2")
            for p in range(NCORES):
                nc.sync.dma_start(hf2[:, p, :], a2a_out[1][p, :, :])
            for st in range(4):
                O = ps.tile([128, 1024], F32, tag="L", name=f"O2_{st}")
                for nn in range(2):
                    n0, n1 = nn * 512, (nn + 1) * 512
                    for kt in range(KT):
                        nc.tensor.matmul(O[:, n0:n1],
                                         hf2[:, kt, st * 128:(st + 1) * 128],
                                         wot[:, kt, n0:n1],
                                         start=(kt == 0), stop=(kt == KT - 1))
                OT = s2p.tile([128, 1024], F32, tag="OT", bufs=2, name=f"OT{st}")
                nc.vector.tensor_add(OT[:, :], O[:, :], o1[:, st, :])
                OG = s2p.tile([128, 1024], F32, tag="OG", bufs=2, name=f"OG{st}")
                nc.scalar.activation(OG[:, :], OT[:, :], GELU)
                nc.sync.dma_start(out_d[st * 128:(st + 1) * 128, :], OG[:, :])

    nc.compile()
    return nc


def _in_maps(q, k, v, Wq, bq, Wk, bk, Wv, bv, Wo, bo):
    xq = [np.ascontiguousarray(q[b].T).astype(BF) for b in range(B)]
    xk = [np.ascontiguousarray(k[b].T).astype(BF) for b in range(B)]
    xv = [np.ascontiguousarray(v[b].T).astype(BF) for b in range(B)]
    wo_bf = np.ascontiguousarray(Wo).astype(BF)
    bo_r = np.asarray(bo).reshape(1, D).astype(BF)
    onr = np.ones((1, 128), BF)
    in_maps = []
    for c in range(NCORES):
        hs = slice(2 * c, 2 * c + 2)
        im = {
            "wq": np.ascontiguousarray(Wq[hs].transpose(1, 0, 2).reshape(D, 128)).astype(BF),
            "wk": np.ascontiguousarray(Wk[hs].transpose(1, 0, 2).reshape(D, 128)).astype(BF),
            "wv": np.ascontiguousarray(Wv[hs].transpose(1, 0, 2).reshape(D, 128)).astype(BF),
            "bq": np.asarray(bq[hs]).reshape(128, 1).astype(np.float32),
            "bk": np.asarray(bk[hs]).reshape(128, 1).astype(np.float32),
            "bv": np.asarray(bv[hs]).reshape(1, 128).astype(BF),
            "wo": wo_bf, "bo": bo_r, "onr": onr,
        }
        for b in range(B):
            im[f"xq{b}"] = xq[b]
            im[f"xk{b}"] = xk[b]
            im[f"xv{b}"] = xv[b]
        in_maps.append(im)
    return in_maps


def kernel(q, k, v, mask, Wq, bq, Wk, bk, Wv, bv, Wo, bo):
    if "nc" not in _CACHE:
        _CACHE["nc"] = _build()
    nc = _CACHE["nc"]
    in_maps = _in_maps(q, k, v, Wq, bq, Wk, bk, Wv, bv, Wo, bo)
    res = run_bass_kernel_spmd(nc, in_maps, core_ids=list(range(NCORES)))
    out = np.empty((B, S, D), np.float32)
    for r in range(NCORES):
        bb, jj = r // 4, r % 4
        out[bb, jj * SQ:(jj + 1) * SQ, :] = res.results[r]["out"]
    return out

